# revision 27
# baseline (speedup 1.0000x reference)
"""nn_DecoderBlock Trainium2 kernel — 8 NeuronCores, token-sharded, fp8.

Self-contained: builds a Bass/Tile SPMD program (one program, all 8
cores; per-core differences are input data), runs it via
run_bass_kernel_spmd, reassembles the full output on the host.

All large GEMMs (QKV/Wo/FFN and the attention score/AV/rowsum matmuls)
run in fp8-e4m3 DoubleRow perf mode (two 128-deep contraction subtiles
per instruction).  Weight tensors are quantized host-side with
per-tensor scales passed in via a `consts` parameter; activations use
fixed power-of-two scales folded into the psum-drain epilogues.  The
softmax keeps exact num/denom scale cancellation (exp bias carries
-ln(16) so p fits fp8).  RMSNorm is deferred past the x-transpose by
folding rs(t) into the rope cos/sin tables; LayerNorm is pipelined
per token block behind the Wo matmul.
"""


import math
from contextlib import ExitStack

import numpy as np
import ml_dtypes

import concourse.bass as bass
import concourse.mybir as mybir
from concourse.tile import TileContext
from concourse.masks import make_identity

try:
    from tile_patch import split_excess_waits
except ImportError:  # self-contained kernel.py defines it later in-file
    pass

F32 = mybir.dt.float32
BF16 = mybir.dt.bfloat16
F8 = mybir.dt.float8e4
AF = mybir.ActivationFunctionType
ALU = mybir.AluOpType
AX = mybir.AxisListType
DR = mybir.MatmulPerfMode.DoubleRow

NEG = -1.0e9
CORES = 8
GPC = 4

# fixed activation scales (power of two; descale folded into epilogues)
S_X = 32.0
S_Q = 256.0
S_K = 32.0
S_V = 32.0
S_CTX = 32.0
S_H2 = 16.0
S_U = 64.0
S_S = 32.0
S_P = 16.0          # p = exp(s - ln S_P); cancels in av/l
CEXP = 1.0 / (S_Q * S_K)
LOG_SP = math.log(S_P)
FP8MAX = 240.0


def full_cfg():
    return dict(B=2, T=2048, D=2048, H=16, DFF=4096)


def mid_cfg():
    return dict(B=2, T=1024, D=2048, H=16, DFF=4096)


def derived(cfg):
    B, T, D, H, DFF = cfg["B"], cfg["T"], cfg["D"], cfg["H"], cfg["DFF"]
    HD = D // H
    assert HD == 128
    TOK = B * T // CORES
    assert T // GPC == TOK and TOK % 128 == 0
    NT = TOK // 128
    assert NT % 2 == 0 and H % 2 == 0
    return dict(HD=HD, TOK=TOK, NT=NT, KD=D // 128, KF=DFF // 128,
                NKB=T // 128, KP=D // 256, KFP=DFF // 256, NTP=NT // 2,
                HP=H // 2)


def build(nc: bass.Bass, cfg):
    B, T, D, H, DFF = cfg["B"], cfg["T"], cfg["D"], cfg["H"], cfg["DFF"]
    dv = derived(cfg)
    TOK, NT, KD, KF, NKB = dv["TOK"], dv["NT"], dv["KD"], dv["KF"], dv["NKB"]
    KP, KFP, NTP, HP = dv["KP"], dv["KFP"], dv["NTP"], dv["HP"]
    DCH = 512
    NDC = D // DCH
    RMS_EPS = float(np.finfo(np.float32).eps)
    LN_EPS = 1e-5
    NJ = GPC - 1
    # gather payload layout: k head-pair tiles then v token-pair tiles
    KCHK = 128 * TOK              # one [128, TOK] fp8 k head tile
    VCHK = 128 * 2 * D            # one [128, 2, D] fp8 v token-pair tile
    VOFF = H * KCHK
    SND = VOFF + NTP * VCHK

    x_in = nc.declare_dram_parameter("x", [TOK, D], F32, isOutput=False)
    wq8 = nc.declare_dram_parameter("wq8", [D, D], F8, isOutput=False)
    wk8 = nc.declare_dram_parameter("wk8", [D, D], F8, isOutput=False)
    wv8 = nc.declare_dram_parameter("wv8", [D, D], F8, isOutput=False)
    wo8 = nc.declare_dram_parameter("wo8", [D, D], F8, isOutput=False)
    w18 = nc.declare_dram_parameter("w18", [D, DFF], F8, isOutput=False)
    wg18 = nc.declare_dram_parameter("wg18", [DFF, DFF], F8, isOutput=False)
    wg28 = nc.declare_dram_parameter("wg28", [DFF, DFF], F8, isOutput=False)
    w28 = nc.declare_dram_parameter("w28", [DFF, D], F8, isOutput=False)
    consts_d = nc.declare_dram_parameter("consts", [128, 8], F32,
                                         isOutput=False)
    b1_d = nc.declare_dram_parameter("b1p", [DFF], F32, isOutput=False)
    bg1_d = nc.declare_dram_parameter("bg1", [DFF], F32, isOutput=False)
    bg2_d = nc.declare_dram_parameter("bg2", [DFF], F32, isOutput=False)
    bo_rep_d = nc.declare_dram_parameter("bo_rep", [128, D], BF16,
                                         isOutput=False)
    b2_rep_d = nc.declare_dram_parameter("b2_rep", [128, D], BF16,
                                         isOutput=False)
    cos_d = nc.declare_dram_parameter("cosT", [128, TOK], F32, isOutput=False)
    sin_d = nc.declare_dram_parameter("sinT", [128, TOK], F32, isOutput=False)
    keybias_d = nc.declare_dram_parameter("keybias", [T], F32, isOutput=False)
    kbown_d = nc.declare_dram_parameter("keybias_own", [TOK], F32,
                                        isOutput=False)
    tri_d = nc.declare_dram_parameter("triT", [128, 128], F32, isOutput=False)
    out_d = nc.declare_dram_parameter("out", [TOK, D], F32, isOutput=True)

    with TileContext(nc) as tc, ExitStack() as top:
        constp = top.enter_context(tc.tile_pool(name="constp", bufs=1))
        dramp = top.enter_context(tc.tile_pool(name="dramp", bufs=1,
                                               space="DRAM"))
        wsp = top.enter_context(tc.tile_pool(name="wsp", bufs=16))
        x2p = top.enter_context(tc.tile_pool(name="x2p", bufs=1))

        # ---- x input first: its DMAs must not queue behind prefetches
        xts = [x2p.tile([128, D], F32, name=f"xt_{t}") for t in range(NT)]
        for t in range(NT):
            for c in range(NDC):
                nc.scalar.dma_start(
                    xts[t][:, c * DCH:(c + 1) * DCH],
                    x_in[t * 128:(t + 1) * 128, c * DCH:(c + 1) * DCH])

        # ---- constants
        ident = constp.tile([128, 128], BF16, name="ident")
        make_identity(nc, ident[:])
        ones_pair = constp.tile([128, 2, 128], F8, name="ones_pair")
        nc.vector.memset(ones_pair[:], 1.0)
        ones_row = constp.tile([1, 128], BF16, name="ones_row")
        nc.vector.memset(ones_row[:], 1.0)
        ones_row_f = constp.tile([1, 128], F32, name="ones_row_f")
        nc.vector.memset(ones_row_f[:], 1.0)
        tri = constp.tile([128, 128], F32, name="tri")
        nc.sync.dma_start(tri[:], tri_d[:])
        cosT = constp.tile([128, TOK], F32, name="cosT")
        sinT = constp.tile([128, TOK], F32, name="sinT")
        nc.sync.dma_start(cosT[:], cos_d[:])
        nc.sync.dma_start(sinT[:], sin_d[:])
        kb_bias = constp.tile([128, NKB], F32, name="kb_bias")
        nc.sync.dma_start(kb_bias[:], keybias_d[:].rearrange("(n p) -> p n",
                                                             p=128))
        kbo_bias = constp.tile([128, NT], F32, name="kbo_bias")
        nc.sync.dma_start(kbo_bias[:], kbown_d[:].rearrange("(n p) -> p n",
                                                            p=128))
        consts = constp.tile([128, 8], F32, name="consts")
        nc.sync.dma_start(consts[:], consts_d[:])
        b1t = constp.tile([128, KF], F32, name="b1t")
        nc.sync.dma_start(b1t[:], b1_d[:].rearrange("(n p) -> p n", p=128))
        bg1t = constp.tile([128, KF], F32, name="bg1t")
        nc.sync.dma_start(bg1t[:], bg1_d[:].rearrange("(n p) -> p n", p=128))
        bg2t = constp.tile([128, KF], F32, name="bg2t")
        nc.sync.dma_start(bg2t[:], bg2_d[:].rearrange("(n p) -> p n", p=128))
        bo_rep = constp.tile([128, D], BF16, name="bo_rep")
        nc.sync.dma_start(bo_rep[:], bo_rep_d[:])
        b2_rep = constp.tile([128, D], BF16, name="b2_rep")
        nc.sync.dma_start(b2_rep[:], b2_rep_d[:])

        snd_k = dramp.tile([VOFF], F8, name="snd_k")
        snd_v = dramp.tile([SND - VOFF], F8, name="snd_v")
        gat_k = dramp.tile([GPC, VOFF], F8, name="gat_k")
        gat_v = dramp.tile([GPC, SND - VOFF], F8, name="gat_v")
        lB_d = dramp.tile([H * TOK], F32, name="lB_d")
        rs_d = dramp.tile([TOK], F32, name="rs_d")

        kpre = {}
        for kp in range(KP):
            wt = x2p.tile([128, 2, DCH], F8, name=f"kw0_{kp}")
            nc.sync.dma_start(
                wt[:],
                wk8[2 * kp * 128:(2 * kp + 2) * 128, 0:DCH]
                .rearrange("(s p) m -> p s m", p=128))
            kpre[(0, kp)] = wt

        x2_t = [x2p.tile([128, D], F32, name=f"x2_{t}") for t in range(NT)]
        ssm = [x2p.tile([128, NDC], F32, name=f"ssm_{t}") for t in range(NT)]
        ssq = [x2p.tile([128, NDC], F32, name=f"ssq_{t}") for t in range(NT)]
        rs_v = [x2p.tile([128, 1], F32, name=f"rsv_{t}") for t in range(NT)]
        cosP = x2p.tile([128, TOK], F32, name="cosP")
        sinP = x2p.tile([128, TOK], F32, name="sinP")

        with tc.tile_pool(name="ctxp", bufs=1) as ctxp:
            ctx2 = [ctxp.tile([128, 2, TOK], F8, name=f"ctx2_{p}")
                    for p in range(KP)]

            with tc.tile_pool(name="xTp", bufs=1) as xTp:
                xT2 = [xTp.tile([128, 2, TOK], F8, name=f"xT2_{p}")
                       for p in range(KP)]

                # ===== phase 1: x transpose -> xT2 (fp8), RMS stats -> rs
                with tc.tile_pool(name="ph1w", bufs=2) as ph1w, \
                     tc.tile_pool(name="ps1", bufs=4, space="PSUM") as ps1:
                    for t in range(NT):
                        xb = ph1w.tile([128, D], BF16, name="xb", tag="xb",
                                       bufs=2)
                        nc.scalar.activation(xb[:], xts[t][:], AF.Copy,
                                             scale=S_X)
                        ss = ph1w.tile([128, NDC], F32, name="ss", tag="ss")
                        sq = ph1w.tile([128, DCH], F32, name="sq", tag="sq")
                        for c in range(NDC):
                            nc.scalar.activation(
                                sq[:], xts[t][:, c * DCH:(c + 1) * DCH],
                                AF.Square, accum_out=ss[:, c:c + 1])
                        ssum = ph1w.tile([128, 1], F32, name="ssum", tag="ssum")
                        nc.vector.tensor_reduce(ssum[:], ss[:], axis=AX.X,
                                                op=ALU.add)
                        nc.vector.tensor_scalar(
                            ssum[:], ssum[:], 1.0 / D, RMS_EPS,
                            op0=ALU.mult, op1=ALU.add)
                        nc.scalar.sqrt(ssum[:], ssum[:])
                        rs = ph1w.tile([128, 1], F32, name="rs", tag="rs")
                        nc.vector.reciprocal(rs[:], ssum[:])
                        nc.vector.tensor_mul(rs_v[t][:], rs[:], consts[:, 2:3])
                        nc.scalar.dma_start(
                            rs_d[t * 128:(t + 1) * 128]
                            .rearrange("(p o) -> p o", o=1), rs[:])
                        for k in range(KD):
                            tp = ps1.tile([128, 128], BF16, name="tp", tag="tp")
                            nc.tensor.transpose(
                                tp[:], xb[:, k * 128:(k + 1) * 128], ident[:])
                            nc.scalar.copy(
                                xT2[k // 2][:, k % 2, t * 128:(t + 1) * 128],
                                tp[:])
                    # rs row replication: DRAM roundtrip -> [1,TOK] ->
                    # f32 matmul replication -> cosP/sinP
                    rsrow = ph1w.tile([1, TOK], F32, name="rsrow", tag="rsrow")
                    nc.scalar.dma_start(
                        rsrow[:], rs_d[:].rearrange("(o t) -> o t", o=1))
                    rsrep = ps1.tile([128, TOK], F32, name="rsrep",
                                     tag="rsrep", bufs=1)
                    nc.tensor.matmul(rsrep[:], ones_row_f[:], rsrow[:],
                                     start=True, stop=True)
                    nc.vector.tensor_mul(cosP[:], cosT[:], rsrep[:])
                    nc.vector.tensor_mul(sinP[:], sinT[:], rsrep[:])

                with tc.tile_pool(name="qkvp", bufs=1) as qkvp:
                    qrT = [qkvp.tile([128, TOK], F8, name=f"qrT_{h}")
                           for h in range(H)]
                    krT = [qkvp.tile([128, TOK], F8, name=f"krT_{h}")
                           for h in range(H)]
                    v2 = [qkvp.tile([128, 2, D], F8, name=f"v2_{p}")
                          for p in range(NTP)]

                    # ===== phase 2: projections (fp8 DR) + rope + gather
                    with tc.tile_pool(name="ph2w", bufs=4) as ph2w, \
                         tc.tile_pool(name="ps2", bufs=2, space="PSUM") as ps2:

                        def rope(dst, src):
                            # src: [128,TOK] bf16 (evens 0:64, odds 64:128)
                            t1 = ph2w.tile([64, TOK], F32, name="rp1",
                                           tag="rp1", bufs=2)
                            t2 = ph2w.tile([64, TOK], F32, name="rp2",
                                           tag="rp2", bufs=2)
                            t3 = ph2w.tile([64, TOK], F32, name="rp3",
                                           tag="rp3", bufs=2)
                            t4 = ph2w.tile([64, TOK], F32, name="rp4",
                                           tag="rp4", bufs=2)
                            nc.vector.tensor_mul(t1[:], src[0:64, :],
                                                 cosP[0:64, :])
                            nc.vector.tensor_mul(t2[:], src[64:128, :],
                                                 sinP[64:128, :])
                            nc.vector.tensor_sub(dst[0:64, :],
                                                 t1[:], t2[:])
                            nc.vector.tensor_mul(t3[:], src[0:64, :],
                                                 sinP[0:64, :])
                            nc.vector.tensor_mul(t4[:], src[64:128, :],
                                                 cosP[64:128, :])
                            nc.vector.tensor_add(dst[64:128, :],
                                                 t3[:], t4[:])

                        def proj_fmajor(wten, cidx, dstl, send, pre=None):
                            for mb in range(KD // 4):
                                psl = [ps2.tile([128, TOK], F32, name=f"mm{m}",
                                                tag=f"mm{m}") for m in range(4)]
                                for kp in range(KP):
                                    if pre is not None and (mb, kp) in pre:
                                        wt = pre[(mb, kp)]
                                    else:
                                        wt = wsp.tile([128, 2, DCH], F8,
                                                      name="wt", tag="w")
                                        nc.sync.dma_start(
                                            wt[:],
                                            wten[2 * kp * 128:
                                                 (2 * kp + 2) * 128,
                                                 mb * DCH:(mb + 1) * DCH]
                                            .rearrange("(s p) m -> p s m",
                                                       p=128))
                                    for m in range(4):
                                        nc.tensor.matmul(
                                            psl[m][:],
                                            wt[:, :, m * 128:(m + 1) * 128],
                                            xT2[kp][:], start=(kp == 0),
                                            stop=(kp == KP - 1), perf_mode=DR)
                                for m in range(4):
                                    h = mb * 4 + m
                                    raw = ph2w.tile([128, TOK], BF16,
                                                    name="rawqk", tag="rawqk")
                                    nc.scalar.activation(
                                        raw[:], psl[m][:], AF.Copy,
                                        scale=consts[:, cidx:cidx + 1])
                                    rope(dstl[h], raw[:])
                            if send and mb == KD // 4 - 1:
                                for hh in range(H):
                                    nc.scalar.dma_start(
                                        snd_k[hh * KCHK:(hh + 1) * KCHK]
                                        .rearrange("(p t) -> p t", p=128),
                                        dstl[hh][:])

                        # k first so its payload is ready earliest
                        with nc.named_scope("kproj"):
                            proj_fmajor(wk8, 0, krT, True, pre=kpre)
                        with nc.named_scope("kgather"):
                            nc.gpsimd.collective_compute(
                                "AllGather", ALU.bypass,
                                replica_groups=[[0, 1, 2, 3], [4, 5, 6, 7]],
                                ins=[snd_k[:]], outs=[gat_k[:]])

                        # v token-major
                        for nd in range(NDC):
                            psl = [ps2.tile([128, DCH], F32, name=f"mm{t}",
                                            tag=f"mm{t}") for t in range(NT)]
                            for kp in range(KP):
                                wt = wsp.tile([128, 2, DCH], F8, name="wt",
                                              tag="w")
                                nc.sync.dma_start(
                                    wt[:],
                                    wv8[2 * kp * 128:(2 * kp + 2) * 128,
                                        nd * DCH:(nd + 1) * DCH]
                                    .rearrange("(s p) m -> p s m", p=128))
                                for t in range(NT):
                                    nc.tensor.matmul(
                                        psl[t][:],
                                        xT2[kp][:, :, t * 128:(t + 1) * 128],
                                        wt[:], start=(kp == 0),
                                        stop=(kp == KP - 1), perf_mode=DR)
                            for t in range(NT):
                                nc.scalar.activation(
                                    v2[t // 2][:, t % 2,
                                               nd * DCH:(nd + 1) * DCH],
                                    psl[t][:], AF.Copy, scale=rs_v[t][:])
                            for tp in range(NTP):
                                nc.scalar.dma_start(
                                    snd_v[tp * VCHK:(tp + 1) * VCHK]
                                    .rearrange("(p s d) -> p s d",
                                               p=128, s=2)
                                    [:, :, nd * DCH:(nd + 1) * DCH],
                                    v2[tp][:, :, nd * DCH:(nd + 1) * DCH])

                        # prefetch first half of q weights ahead of the
                        # gather so its DMA traffic cannot starve q-proj
                        qpre = {}
                        for mb in range(2):
                            for kp in range(KP):
                                wt = ph2w.tile([128, 2, DCH], F8,
                                               name=f"qw{mb}_{kp}",
                                               tag=f"qw{mb}_{kp}", bufs=1)
                                nc.sync.dma_start(
                                    wt[:],
                                    wq8[2 * kp * 128:(2 * kp + 2) * 128,
                                        mb * DCH:(mb + 1) * DCH]
                                    .rearrange("(s p) m -> p s m", p=128))
                                qpre[(mb, kp)] = wt

                        with nc.named_scope("vgather"):
                            nc.gpsimd.collective_compute(
                                "AllGather", ALU.bypass,
                                replica_groups=[[0, 1, 2, 3], [4, 5, 6, 7]],
                                ins=[snd_v[:]], outs=[gat_v[:]])

                        # q last: overlaps the gather
                        with nc.named_scope("qproj"):
                            proj_fmajor(wq8, 1, qrT, False, pre=qpre)

                    # ===== phase 3: attention (fp8 DR)
                    with tc.tile_pool(name="ph3b", bufs=1) as ph3b, \
                         tc.tile_pool(name="ph3w", bufs=3) as ph3w, \
                         tc.tile_pool(name="ps3", bufs=1, space="PSUM") as ps3:
                        ctxB = [ph3b.tile([128, TOK], BF16, name=f"ctxB_{h}")
                                for h in range(H)]
                        lBs = [ph3b.tile([128, TOK], BF16, name=f"lBs_{h}")
                               for h in range(H)]

                        def qk_block(h, sps, lhs_k, bias_ap, p2, s2, diag):
                            nc.tensor.matmul(sps[:], lhs_k, qrT[h][:],
                                             start=True, stop=True)
                            if diag is not None:
                                nc.vector.tensor_add(
                                    sps[:, diag * 128:(diag + 1) * 128],
                                    sps[:, diag * 128:(diag + 1) * 128],
                                    tri[:])
                            nc.scalar.activation(p2[:, s2, :], sps[:], AF.Exp,
                                                 bias=bias_ap, scale=CEXP)
                            if diag is not None and diag > 0:
                                nc.vector.memset(p2[:, s2, 0:diag * 128], 0.0)

                        # part B: own causal diagonal (no gathered data)
                        for h in range(H):
                            avB = ps3.tile([128, TOK], F32, name="avB",
                                           tag="av", bufs=2)
                            lB = ps3.tile([128, TOK], F32, name="lB",
                                           tag="lp", bufs=2)
                            pend_avB = None
                            for tp in range(NTP):
                                p2 = ph3w.tile([128, 2, TOK], F8, name="p2",
                                               tag="p2", bufs=4)
                                for s2 in range(2):
                                    kbl = 2 * tp + s2
                                    sps = ps3.tile([128, TOK], F32, name="sps",
                                                   tag="sps", bufs=3)
                                    qk_block(
                                        h, sps,
                                        krT[h][:, kbl * 128:(kbl + 1) * 128],
                                        kbo_bias[:, kbl:kbl + 1], p2, s2, kbl)
                                if pend_avB is not None:
                                    ptp, pp2, pfirst, plast = pend_avB
                                    nc.tensor.matmul(
                                        lB[:], ones_pair[:], pp2[:],
                                        start=pfirst, stop=plast, perf_mode=DR)
                                    nc.tensor.matmul(
                                        avB[:],
                                        v2[ptp][:, :, h * 128:(h + 1) * 128],
                                        pp2[:], start=pfirst, stop=plast,
                                        perf_mode=DR)
                                pend_avB = (tp, p2, tp == 0, tp == NTP - 1)
                            ptp, pp2, pfirst, plast = pend_avB
                            nc.tensor.matmul(
                                lB[:], ones_pair[:], pp2[:], start=pfirst,
                                stop=plast, perf_mode=DR)
                            nc.tensor.matmul(
                                avB[:], v2[ptp][:, :, h * 128:(h + 1) * 128],
                                pp2[:], start=pfirst, stop=plast, perf_mode=DR)
                            nc.scalar.copy(ctxB[h][:], avB[:])
                            nc.scalar.copy(lBs[h][:], lB[:])

                        # part A: gathered peers (fully masked rows too —
                        # uniform SPMD program, keybias zeroes them).
                        # Epilogue runs one head behind the matmul stream so
                        # the PE never waits on the l roundtrip.
                        def epilogue(h, av, lp):
                            lsum = ph3w.tile([128, TOK], BF16, name="lsum",
                                             tag="lsum", bufs=2)
                            nc.vector.tensor_add(lsum[:], lp[:], lBs[h][:])
                            linv = ph3w.tile([128, TOK], F32, name="linv",
                                             tag="linv", bufs=2)
                            nc.vector.reciprocal(linv[:], lsum[:])
                            avf = ph3w.tile([128, TOK], BF16, name="avf",
                                            tag="avf", bufs=2)
                            nc.vector.tensor_add(avf[:], av[:], ctxB[h][:])
                            nc.vector.tensor_mul(
                                ctx2[h // 2][:, h % 2, :], avf[:], linv[:])

                        pend = None
                        for hp in range(HP):
                            # gathered k for this head pair: one load per
                            # (hp, j), shared by both heads; split into two
                            # DMAs to spread across queues
                            ktbs = []
                            for j in range(NJ):
                                ktb = ph3w.tile([128, 2, TOK], F8, name="ktb",
                                                tag="ktb", bufs=2 * NJ)
                                gsrc = gat_k[j, 2 * hp * KCHK:
                                             (2 * hp + 2) * KCHK] \
                                    .rearrange("(s p t) -> p s t", p=128, s=2)
                                nc.scalar.dma_start(ktb[:, 0, :],
                                                    gsrc[:, 0, :])
                                nc.scalar.dma_start(ktb[:, 1, :],
                                                    gsrc[:, 1, :])
                                ktbs.append(ktb)
                            for h2 in range(2):
                                h = 2 * hp + h2
                                av = ps3.tile([128, TOK], F32, name="av",
                                              tag="av", bufs=2)
                                lp = ps3.tile([128, TOK], F32, name="lp",
                                              tag="lp", bufs=2)
                                pend_av = None
                                for j in range(NJ):
                                    for tp in range(NTP):
                                        vtb = ph3w.tile([128, 2, 128], F8,
                                                        name="vtb", tag="vtb",
                                                        bufs=6)
                                        nc.scalar.dma_start(
                                            vtb[:],
                                            gat_v[j, tp * VCHK:
                                                  (tp + 1) * VCHK]
                                            .rearrange("(p s d) -> p s d",
                                                       p=128, s=2)
                                            [:, :, h * 128:(h + 1) * 128])
                                        p2 = ph3w.tile([128, 2, TOK], F8,
                                                       name="p2", tag="p2",
                                                       bufs=4)
                                        for s2 in range(2):
                                            kbl = 2 * tp + s2
                                            kb = j * NT + kbl
                                            sps = ps3.tile([128, TOK], F32,
                                                           name="sps",
                                                           tag="sps", bufs=3)
                                            qk_block(
                                                h, sps,
                                                ktbs[j][:, h2, kbl * 128:
                                                        (kbl + 1) * 128],
                                                kb_bias[:, kb:kb + 1],
                                                p2, s2, None)
                                        # emit previous iteration's av/lps
                                        # AFTER this one's scores: the PE
                                        # stream never waits on a fresh exp
                                        if pend_av is not None:
                                            pvtb, pp2, pfirst, plast = pend_av
                                            nc.tensor.matmul(
                                                lp[:], ones_pair[:], pp2[:],
                                                start=pfirst, stop=plast,
                                                perf_mode=DR)
                                            nc.tensor.matmul(
                                                av[:], pvtb[:], pp2[:],
                                                start=pfirst, stop=plast,
                                                perf_mode=DR)
                                        first = j == 0 and tp == 0
                                        last = j == NJ - 1 and tp == NTP - 1
                                        pend_av = (vtb, p2, first, last)
                                pvtb, pp2, pfirst, plast = pend_av
                                nc.tensor.matmul(
                                    lp[:], ones_pair[:], pp2[:],
                                    start=pfirst, stop=plast, perf_mode=DR)
                                nc.tensor.matmul(
                                    av[:], pvtb[:], pp2[:], start=pfirst,
                                    stop=plast, perf_mode=DR)
                                if pend is not None:
                                    epilogue(*pend)
                                pend = (h, av, lp)
                        epilogue(*pend)

            # ===== phase 4+5: Wo + residual -> x2, LN pipelined per t
            h2T2 = [x2p.tile([128, 2, TOK], F8, name=f"h2T_{p}")
                    for p in range(KP)]
            with tc.tile_pool(name="ph4w", bufs=3) as ph4w, \
                 tc.tile_pool(name="ps4", bufs=2, space="PSUM") as ps4:

                def ln_block(t):
                    mu_n = ph4w.tile([128, 1], F32, name="mu_n", tag="mu_n")
                    nc.vector.tensor_reduce(mu_n[:], ssm[t][:], axis=AX.X,
                                            op=ALU.add)
                    nc.vector.tensor_scalar(mu_n[:], mu_n[:], -1.0 / D, None,
                                            op0=ALU.mult)
                    var = ph4w.tile([128, 1], F32, name="var", tag="var")
                    nc.vector.tensor_reduce(var[:], ssq[t][:], axis=AX.X,
                                            op=ALU.add)
                    nc.vector.tensor_scalar(var[:], var[:], 1.0 / D, LN_EPS,
                                            op0=ALU.mult, op1=ALU.add)
                    mu2 = ph4w.tile([128, 1], F32, name="mu2", tag="mu2")
                    nc.vector.tensor_mul(mu2[:], mu_n[:], mu_n[:])
                    nc.vector.tensor_sub(var[:], var[:], mu2[:])
                    nc.scalar.sqrt(var[:], var[:])
                    rsl = ph4w.tile([128, 1], F32, name="rsl", tag="rsl")
                    nc.vector.reciprocal(rsl[:], var[:])
                    nc.vector.tensor_scalar(rsl[:], rsl[:], S_H2, None,
                                            op0=ALU.mult)
                    nbi = ph4w.tile([128, 1], F32, name="nbi", tag="nbi")
                    nc.vector.tensor_mul(nbi[:], mu_n[:], rsl[:])
                    h2 = ph4w.tile([128, D], BF16, name="h2", tag="h2",
                                   bufs=2)
                    nc.scalar.activation(h2[:], x2_t[t][:], AF.Identity,
                                         bias=nbi[:], scale=rsl[:])
                    for k in range(KD):
                        tp = ps4.tile([128, 128], BF16, name="tp5", tag="tp5")
                        nc.tensor.transpose(tp[:], h2[:, k * 128:(k + 1) * 128],
                                            ident[:])
                        nc.vector.tensor_copy(
                            h2T2[k // 2][:, k % 2, t * 128:(t + 1) * 128],
                            tp[:])

                for nd in range(NDC):
                    psl = [ps4.tile([128, DCH], F32, name=f"pso{t}",
                                    tag=f"pso{t}", bufs=1) for t in range(NT)]
                    for kp in range(KP):
                        wt = wsp.tile([128, 2, DCH], F8, name="wt", tag="w")
                        nc.sync.dma_start(
                            wt[:],
                            wo8[2 * kp * 128:(2 * kp + 2) * 128,
                                nd * DCH:(nd + 1) * DCH]
                            .rearrange("(s p) m -> p s m", p=128))
                        for t in range(NT):
                            nc.tensor.matmul(
                                psl[t][:],
                                ctx2[kp][:, :, t * 128:(t + 1) * 128],
                                wt[:], start=(kp == 0), stop=(kp == KP - 1),
                                perf_mode=DR)
                    for t in range(NT):
                        tt1 = ph4w.tile([128, DCH], F32, name="tt1", tag="tt1")
                        nc.vector.scalar_tensor_tensor(
                            tt1[:], psl[t][:], consts[:, 3:4],
                            xts[t][:, nd * DCH:(nd + 1) * DCH],
                            op0=ALU.mult, op1=ALU.add)
                        nc.vector.tensor_add(
                            x2_t[t][:, nd * DCH:(nd + 1) * DCH], tt1[:],
                            bo_rep[:, nd * DCH:(nd + 1) * DCH])
                        sqt = ph4w.tile([128, DCH], F32, name="sqt", tag="sqt")
                        nc.scalar.activation(
                            sqt[:], x2_t[t][:, nd * DCH:(nd + 1) * DCH],
                            AF.Square, accum_out=ssq[t][:, nd:nd + 1])
                        nc.vector.tensor_reduce(
                            ssm[t][:, nd:nd + 1],
                            x2_t[t][:, nd * DCH:(nd + 1) * DCH],
                            axis=AX.X, op=ALU.add)
                        if nd == NDC - 1:
                            ln_block(t)

        # ===== phases 6-8: FFN
        with tc.tile_pool(name="ffnp", bufs=1) as ffnp:
            uT2 = [ffnp.tile([128, 2, TOK], F8, name=f"uT_{p}")
                   for p in range(KFP)]
            sT2 = [ffnp.tile([128, 2, TOK], F8, name=f"sT_{p}")
                   for p in range(KFP)]

            with tc.tile_pool(name="ph6w", bufs=2) as ph6w, \
                 tc.tile_pool(name="ps6", bufs=2, space="PSUM") as ps6:
                # W1
                for mb in range(KF // 4):
                    psl = [ps6.tile([128, TOK], F32, name=f"mm{m}",
                                    tag=f"mm{m}") for m in range(4)]
                    for kp in range(KP):
                        wt = wsp.tile([128, 2, DCH], F8, name="wt", tag="w")
                        nc.sync.dma_start(
                            wt[:],
                            w18[2 * kp * 128:(2 * kp + 2) * 128,
                                mb * DCH:(mb + 1) * DCH]
                            .rearrange("(s p) m -> p s m", p=128))
                        for m in range(4):
                            nc.tensor.matmul(
                                psl[m][:], wt[:, :, m * 128:(m + 1) * 128],
                                h2T2[kp][:], start=(kp == 0),
                                stop=(kp == KP - 1), perf_mode=DR)
                    for m in range(4):
                        kf = mb * 4 + m
                        nc.scalar.activation(
                            uT2[kf // 2][:, kf % 2, :], psl[m][:], AF.Identity,
                            bias=b1t[:, kf:kf + 1],
                            scale=consts[:, 4:5])

                # Wg1 (silu) and Wg2, fused per output block group
                for mb in range(KF // 4):
                    psa = [ps6.tile([128, TOK], F32, name=f"mma{m}",
                                    tag=f"mm{m}") for m in range(4)]
                    for kp in range(KFP):
                        wt = wsp.tile([128, 2, DCH], F8, name="wt", tag="w")
                        nc.sync.dma_start(
                            wt[:],
                            wg18[2 * kp * 128:(2 * kp + 2) * 128,
                                 mb * DCH:(mb + 1) * DCH]
                            .rearrange("(s p) m -> p s m", p=128))
                        for m in range(4):
                            nc.tensor.matmul(
                                psa[m][:], wt[:, :, m * 128:(m + 1) * 128],
                                uT2[kp][:], start=(kp == 0),
                                stop=(kp == KFP - 1), perf_mode=DR)
                    sgl = []
                    for m in range(4):
                        kf = mb * 4 + m
                        sg = ph6w.tile([128, TOK], BF16, name=f"sg{m}",
                                       tag=f"sg{m}")
                        nc.scalar.activation(sg[:], psa[m][:], AF.Silu,
                                             bias=bg1t[:, kf:kf + 1],
                                             scale=consts[:, 5:6])
                        sgl.append(sg)
                    psb = [ps6.tile([128, TOK], F32, name=f"mmb{m}",
                                    tag=f"mm{m}") for m in range(4)]
                    for kp in range(KFP):
                        wt = wsp.tile([128, 2, DCH], F8, name="wt", tag="w")
                        nc.sync.dma_start(
                            wt[:],
                            wg28[2 * kp * 128:(2 * kp + 2) * 128,
                                 mb * DCH:(mb + 1) * DCH]
                            .rearrange("(s p) m -> p s m", p=128))
                        for m in range(4):
                            nc.tensor.matmul(
                                psb[m][:], wt[:, :, m * 128:(m + 1) * 128],
                                uT2[kp][:], start=(kp == 0),
                                stop=(kp == KFP - 1), perf_mode=DR)
                    for m in range(4):
                        kf = mb * 4 + m
                        g2v = ph6w.tile([128, TOK], BF16, name="g2v",
                                        tag="g2v")
                        nc.scalar.activation(g2v[:], psb[m][:], AF.Identity,
                                             bias=bg2t[:, kf:kf + 1],
                                             scale=consts[:, 6:7])
                        nc.vector.tensor_mul(
                            sT2[kf // 2][:, kf % 2, :], sgl[m][:], g2v[:])

            # W2 token-major + residual + store
            with tc.tile_pool(name="ph8w", bufs=3) as ph8w, \
                 tc.tile_pool(name="ps8", bufs=2, space="PSUM") as ps8:
                for nd in range(NDC):
                    psl = [ps8.tile([128, DCH], F32, name=f"mm{t}",
                                    tag=f"mm{t}") for t in range(NT)]
                    for kp in range(KFP):
                        wt = wsp.tile([128, 2, DCH], F8, name="wt", tag="w")
                        nc.sync.dma_start(
                            wt[:],
                            w28[2 * kp * 128:(2 * kp + 2) * 128,
                                nd * DCH:(nd + 1) * DCH]
                            .rearrange("(s p) m -> p s m", p=128))
                        for t in range(NT):
                            nc.tensor.matmul(
                                psl[t][:],
                                sT2[kp][:, :, t * 128:(t + 1) * 128],
                                wt[:], start=(kp == 0), stop=(kp == KFP - 1),
                                perf_mode=DR)
                    for t in range(NT):
                        tt1 = ph8w.tile([128, DCH], F32, name="o1", tag="o1")
                        nc.vector.scalar_tensor_tensor(
                            tt1[:], psl[t][:], consts[:, 7:8],
                            x2_t[t][:, nd * DCH:(nd + 1) * DCH],
                            op0=ALU.mult, op1=ALU.add)
                        yf = ph8w.tile([128, DCH], F32, name="yf", tag="yf")
                        nc.vector.tensor_add(
                            yf[:], tt1[:], b2_rep[:, nd * DCH:(nd + 1) * DCH])
                        nc.sync.dma_start(
                            out_d[t * 128:(t + 1) * 128,
                                  nd * DCH:(nd + 1) * DCH], yf[:])
    split_excess_waits(nc)
    return nc


# ---------------------------------------------------------------- host side


def _q8(w, s):
    return np.clip(np.asarray(w, np.float32) * s, -FP8MAX, FP8MAX).astype(
        ml_dtypes.float8_e4m3)


def host_prepare(inputs, cfg):
    B, T, D, H, DFF = cfg["B"], cfg["T"], cfg["D"], cfg["H"], cfg["DFF"]
    dv = derived(cfg)
    HD, TOK = dv["HD"], dv["TOK"]
    f32 = np.float32
    bf = ml_dtypes.bfloat16

    x = np.asarray(inputs["x"], f32)
    g_rms = np.asarray(inputs["g_rms"], f32)
    g_ln = np.asarray(inputs["g_ln"], f32)
    b_ln = np.asarray(inputs["b_ln"], f32)
    pad = np.asarray(inputs["pad_mask"])

    if np.abs(np.asarray(inputs["bq"], f32)).max() != 0 or \
       np.abs(np.asarray(inputs["bk"], f32)).max() != 0:
        raise NotImplementedError("nonzero bq/bk not supported by this kernel")

    perm = np.concatenate(
        [h * HD + np.concatenate([np.arange(0, HD, 2), np.arange(1, HD, 2)])
         for h in range(H)])
    wq = (g_rms[:, None] * np.asarray(inputs["Wq"], f32))[:, perm]
    wk = (g_rms[:, None] * np.asarray(inputs["Wk"], f32))[:, perm]
    wv = g_rms[:, None] * np.asarray(inputs["Wv"], f32)
    wo = np.asarray(inputs["Wo"], f32)
    w1 = g_ln[:, None] * np.asarray(inputs["W1"], f32)
    wg1 = np.asarray(inputs["Wg1"], f32)
    wg2 = np.asarray(inputs["Wg2"], f32)
    w2 = np.asarray(inputs["W2"], f32)

    def ws(w):
        m = np.abs(w).max()
        return (FP8MAX - 16.0) / m if m > 0 else 1.0

    s_wq, s_wk, s_wv, s_wo = ws(wq), ws(wk), ws(wv), ws(wo)
    s_w1, s_wg1, s_wg2, s_w2 = ws(w1), ws(wg1), ws(wg2), ws(w2)
    wq8 = _q8(wq, s_wq)
    wk8 = _q8(wk, s_wk)
    wv8 = _q8(wv, s_wv)
    wo8 = _q8(wo, s_wo)
    w18 = _q8(w1, s_w1)
    wg18 = _q8(wg1, s_wg1)
    wg28 = _q8(wg2, s_wg2)
    w28 = _q8(w2, s_w2)

    qscale = 1.0 / math.sqrt(HD)
    cvals = np.array([
        S_K / (s_wk * S_X),
        S_Q * qscale / (s_wq * S_X),
        S_V / (s_wv * S_X),
        1.0 / (s_wo * S_CTX),
        S_U / (s_w1 * S_H2),
        1.0 / (s_wg1 * S_U),
        S_S / (s_wg2 * S_U),
        1.0 / (s_w2 * S_S),
    ], f32)
    consts = np.broadcast_to(cvals[None, :], (128, 8)).copy()

    b1p = (S_U * (np.asarray(inputs["b1"], f32)
                  + b_ln @ np.asarray(inputs["W1"], f32))).astype(f32)
    bg1 = np.asarray(inputs["bg1"], f32)
    bg2 = (S_S * np.asarray(inputs["bg2"], f32)).astype(f32)
    bo_rep = np.broadcast_to(np.asarray(inputs["bo"], f32),
                             (128, D)).astype(bf)
    b2_rep = np.broadcast_to(np.asarray(inputs["b2"], f32),
                             (128, D)).astype(bf)

    inv_freq = 1.0 / (10000.0 ** (np.arange(0, HD, 2, dtype=f32) / HD))
    ang = np.arange(T, dtype=f32)[:, None] * inv_freq[None, :]
    cosA, sinA = np.cos(ang).astype(f32), np.sin(ang).astype(f32)

    tri = np.where(np.arange(128)[:, None] <= np.arange(128)[None, :],
                   np.float32(0.0), np.float32(NEG))

    in_maps = []
    for i in range(CORES):
        g, p = i // GPC, i % GPC
        t0 = p * TOK
        kb = np.where(pad[g] == 0, np.float32(NEG),
                      np.float32(-LOG_SP))
        kb[t0:] = NEG
        kbo = np.where(pad[g, t0:t0 + TOK] == 0, np.float32(NEG),
                       np.float32(-LOG_SP))
        in_maps.append(dict(
            x=np.ascontiguousarray(x[g, t0:t0 + TOK]),
            wq8=wq8, wk8=wk8, wv8=wv8, wo8=wo8, w18=w18, wg18=wg18,
            wg28=wg28, w28=w28, consts=consts,
            b1p=b1p, bg1=bg1, bg2=bg2, bo_rep=bo_rep, b2_rep=b2_rep,
            cosT=np.ascontiguousarray(
                np.tile(cosA[t0:t0 + TOK].T, (2, 1))),
            sinT=np.ascontiguousarray(
                np.tile(sinA[t0:t0 + TOK].T, (2, 1))),
            keybias=kb, keybias_own=kbo, triT=tri,
        ))
    return in_maps


def host_assemble(results, cfg):
    B, T, D = cfg["B"], cfg["T"], cfg["D"]
    TOK = derived(cfg)["TOK"]
    out = np.empty((B, T, D), np.float32)
    for i in range(CORES):
        g, p = i // GPC, i % GPC
        out[g, p * TOK:(p + 1) * TOK] = results[i]["out"]
    return out


# ---------------------------------------------------------------- numpy ref


def numpy_reference(inputs, cfg):
    B, T, D, H, DFF = cfg["B"], cfg["T"], cfg["D"], cfg["H"], cfg["DFF"]
    HD = D // H
    f = np.float32
    x = np.asarray(inputs["x"], f)
    RMS_EPS = float(np.finfo(np.float32).eps)

    h = x * (1.0 / np.sqrt((x * x).mean(-1, keepdims=True) + RMS_EPS))
    h = h * inputs["g_rms"]
    q = (h @ inputs["Wq"] + inputs["bq"]).reshape(B, T, H, HD).transpose(0, 2, 1, 3)
    k = (h @ inputs["Wk"] + inputs["bk"]).reshape(B, T, H, HD).transpose(0, 2, 1, 3)
    v = (h @ inputs["Wv"]).reshape(B, T, H, HD).transpose(0, 2, 1, 3)

    inv_freq = 1.0 / (10000.0 ** (np.arange(0, HD, 2, dtype=f) / HD))
    ang = np.arange(T, dtype=f)[:, None] * inv_freq[None, :]
    cos, sin = np.cos(ang), np.sin(ang)

    def rope(z):
        z1, z2 = z[..., ::2], z[..., 1::2]
        out = np.stack([z1 * cos - z2 * sin, z1 * sin + z2 * cos], -1)
        return out.reshape(z.shape)

    q, k = rope(q), rope(k)
    scores = np.einsum("bhqd,bhkd->bhqk", q, k) / np.sqrt(np.float32(HD))
    causal = np.tril(np.ones((T, T), bool))
    mask = (np.asarray(inputs["pad_mask"])[:, None, :].astype(bool)
            & causal)[:, None]
    scores = np.where(mask, scores, -np.inf)
    m = scores.max(-1, keepdims=True)
    e = np.exp(scores - m)
    attn = e / e.sum(-1, keepdims=True)
    o = np.einsum("bhqk,bhkd->bhqd", attn, v)
    o = o.transpose(0, 2, 1, 3).reshape(B, T, D)
    x = x + o @ inputs["Wo"] + inputs["bo"]

    mu = x.mean(-1, keepdims=True)
    var = ((x - mu) ** 2).mean(-1, keepdims=True)
    h2 = (x - mu) / np.sqrt(var + 1e-5) * inputs["g_ln"] + inputs["b_ln"]
    u = h2 @ inputs["W1"] + inputs["b1"]
    g1 = u @ inputs["Wg1"] + inputs["bg1"]
    s = (g1 / (1 + np.exp(-g1))) * (u @ inputs["Wg2"] + inputs["bg2"])
    return x + s @ inputs["W2"] + inputs["b2"]


def make_small_inputs(cfg, seed=0):
    B, T, D, H, DFF = cfg["B"], cfg["T"], cfg["D"], cfg["H"], cfg["DFF"]
    rng = np.random.default_rng(seed)
    f = np.float32

    def w(shape, fan):
        return ((rng.random(shape, dtype=f) * 2 - 1) / np.sqrt(fan)).astype(f)

    lengths = rng.integers(T // 2, T + 1, size=(B,))
    pad = (np.arange(T)[None, :] < lengths[:, None]).astype(np.int32)
    return dict(
        x=rng.standard_normal((B, T, D), dtype=f),
        Wq=w((D, D), D), bq=np.zeros(D, f),
        Wk=w((D, D), D), bk=np.zeros(D, f),
        Wv=w((D, D), D),
        Wo=w((D, D), D), bo=rng.standard_normal(D, dtype=f) * 0.02,
        W1=w((D, DFF), D), b1=rng.standard_normal(DFF, dtype=f) * 0.02,
        Wg1=w((DFF, DFF), DFF), bg1=rng.standard_normal(DFF, dtype=f) * 0.02,
        Wg2=w((DFF, DFF), DFF), bg2=rng.standard_normal(DFF, dtype=f) * 0.02,
        W2=w((DFF, D), DFF), b2=rng.standard_normal(D, dtype=f) * 0.02,
        g_rms=(1 + 0.1 * rng.standard_normal(D)).astype(f),
        g_ln=(1 + 0.1 * rng.standard_normal(D)).astype(f),
        b_ln=(0.05 * rng.standard_normal(D)).astype(f),
        pad_mask=pad,
    )


# ===================== tile scheduler patch =====================


import concourse.tile as tile


def _split_drain_and_barrier(self, tick_clock, wait_clock):
    from concourse.vector_clock import ScopedClock

    drain_inst = self.nc.sync.drain()
    wait_clock.add_sem_waits(
        drain_inst.ins, ScopedClock({None: tick_clock.global_clock})
    )
    si = drain_inst.ins.sync_info
    waits = list(si.on_wait) if si and si.on_wait else []
    if len(waits) > 1:
        si.on_wait.clear()
        si.on_wait.extend(waits[:1])
        for i in range(1, len(waits), 1):
            extra = self.nc.sync.drain()
            esi = extra.ins.sync_info
            if esi is None:
                import concourse.mybir as mybir

                extra.ins.sync_info = mybir.SyncInfo(
                    on_wait=waits[i : i + 1], on_update=[]
                )
            else:
                esi.on_wait.extend(waits[i : i + 1])

    self.nc.all_engine_barrier()
    assert self.sems is not None
    popped = self.nc._tile_sem_poison_stack.pop()
    assert popped is self._sem_poison
    self.nc.clear_and_free_semaphores(list(self.sems.allocated().values()))
    self.nc.all_engine_barrier()


def split_excess_waits(nc, default_limit=1, ctrl_limit=1, dma_limit=1):
    """Walrus in this container rejects instructions whose sync_info
    carries more wait commands than the ISA encoding has slots for.
    Move excess waits onto same-engine no-op carriers inserted right
    before the offending instruction (engine queues are in-order, so the
    carrier's waits are observed before the instruction issues)."""
    import concourse.mybir as mybir

    CTRL = ("InstDrain", "InstNoOp", "InstEventSemaphore")
    DMA = ("InstDMACopy", "InstTriggeredCopy", "InstDMATranspose")
    nsplit = 0
    for bb_name, bbw in list(nc.bb_map.items()):
        bb = bbw.bb if hasattr(bbw, "bb") else bbw
        insts = bb.instructions
        i = 0
        while i < len(insts):
            inst = insts[i]
            tname = type(inst).__name__
            limit = (ctrl_limit if tname in CTRL
                     else dma_limit if tname in DMA else default_limit)
            si = inst.sync_info
            waits = list(si.on_wait) if si and si.on_wait else []
            if len(waits) > limit:
                keep, extra = waits[:limit], waits[limit:]
                si.on_wait.clear()
                si.on_wait.extend(keep)
                ncar = 0
                for j in range(0, len(extra), ctrl_limit):
                    chunk = extra[j:j + ctrl_limit]
                    car = nc.engines[inst.engine].nop(nofuse=True).ins
                    # nop() appended to the current bb; move it here
                    for other in nc.bb_map.values():
                        obb = other.bb if hasattr(other, "bb") else other
                        if obb.instructions and obb.instructions[-1] is car:
                            obb.instructions.pop()
                            break
                    car.sync_info = mybir.SyncInfo(on_wait=chunk, on_update=[])
                    insts.insert(i, car)
                    ncar += 1
                i += ncar
                nsplit += 1
            i += 1
    return nsplit


def _apply_tile_patch():
    tile.TileContext._drain_and_barrier = _split_drain_and_barrier


# ================================================================ runner

_tile_patch_applied = False
_build_cache = {}
LAST_EXEC_NS = None


def _get_nc():
    global _tile_patch_applied
    if not _tile_patch_applied:
        _apply_tile_patch()
        _tile_patch_applied = True
    if "nc" not in _build_cache:
        nc = bass.Bass()
        build(nc, full_cfg())
        _build_cache["nc"] = nc
    return _build_cache["nc"]


def kernel(_profile=False, **inputs):
    """Full-input decoder block on 8 TRN2 NeuronCores.

    inputs: the arrays from reference.setup_inputs() (numpy or jax).
    Returns the full [B, T, D] float32 output.
    """
    global LAST_EXEC_NS
    from concourse.bass_utils import run_bass_kernel_spmd

    cfg = full_cfg()
    nc = _get_nc()
    in_maps = host_prepare({k: np.asarray(v) for k, v in inputs.items()}, cfg)
    res = run_bass_kernel_spmd(nc, in_maps, list(range(CORES)),
                               trace=bool(_profile))
    LAST_EXEC_NS = getattr(res, "exec_time_ns", None)
    return host_assemble(res.results, cfg)
